# revision 25
# baseline (speedup 1.0000x reference)
"""Deformable Conv1D kernel v12 for Trainium2 (8 NeuronCores, Bass/Tile).

Diagonal (banded) layout. y[i] = sum_{d,k} W[3-k] * x[i+d] *
hat(off[i+d] + (2 - d) - k), where off = relu(conv1d(x)) - x and
hat(u) = relu(min(u+1, 1-u)); the band d = j - i is in [DMIN, DMAX]
(covers supp of the hat for the actual off range, asserted on host).

Per core (512 outputs, split into 2 halves h of 256):
  rows p = (k, d, h), NP = 3*ND*2 = 78 partitions; free dim = ii (256).

  psC[p,ii] = conv(x)[j],  j = 512*core + 256*h + d + ii     (one PE matmul:
              lhsT = block conv weights [NR, NP], rhs = shifted/batch-masked
              x rows [NR, 256], both bf16)
  H[p,ii]   = x[j] * relu(min(off + s0[p], s1[p] - off)),
              off = relu(psC) - x[j]                          (ONE fused
              custom-DVE uop straight from PSUM; s0 = 3-k-d, s1 = k-1+d)
  psY[h,ii] = sum_p WL[p,h] * H[p,ii]                         (one PE matmul,
              f32r full-rate at 256 cols; WL[p,h'] = W[3-k(p)]*[h(p)==h'])
  y         = psY -> SBUF copy -> DMA out.

Input schedule: two DMAs, each FIRST on its own HWDGE queue (the second
DMA on a queue starts its packets ~1us later): PK13 = conv rhs+lhsT in
bf16, 672B rows, on the Sync queue; PK2 = fp32 x values + W column +
hat scalars, 1040B rows, on the Activation queue.  Rows are kept
<= 1KB — larger rows collapse onto a single SDMA engine and serialize.
Dummy matmuls with no input deps run during the DMA wait to ramp the
PE p-state.  Everything else (preamble barriers, DMA trigger latency
~1.5us, and the NRT-injected 253-semaphore clear epilogue ~6us) is
fixed harness overhead.
"""

import sys

for _p in ("/opt/trn_rl_repo",):
    if _p not in sys.path:
        sys.path.insert(0, _p)

import numpy as np

import concourse.bass as bass
import concourse.tile as tile
from concourse import bacc, mybir
from concourse import dve_ops as _dve_ops
from concourse.bass_utils import run_bass_kernel_spmd
from concourse.dve_ops import DveOp
from concourse.dve_spec import C0, C1, Spec, Src0, Src1, minn, relu

# Fused custom-DVE op (single uop):
#   off = relu(in0) - in1;  out = in1 * relu(min(off + s0, s1 - off))
_r = relu(Src0)
_t = _r - Src1
HATX = DveOp(
    "HATX",
    Spec(
        body=Src1 * relu(minn(_t + C0, C1 - _t)),
        reference=lambda in0, in1, s0, s1, imm2: (
            lambda off: (
                in1 * np.maximum(np.minimum(off + s0, s1 - off), 0)
            ).astype(np.float32)
        )(np.maximum(in0, 0) - in1),
    ),
    subdim=False,
    uops_sha={"v3": "355cfd13d7758520", "v4": "aaff841bf6e5b216"},
)


def _register(op):
    if op.name not in _dve_ops._SUB_OPCODE_FOR_NAME:
        _dve_ops.OPS.append(op)
        _dve_ops.CUSTOM_DVE_SPECS[op.name] = op.spec
        _dve_ops._SUB_OPCODE_FOR_NAME[op.name] = (
            max(_dve_ops._SUB_OPCODE_FOR_NAME.values()) + 1)
        assert _dve_ops._SUB_OPCODE_FOR_NAME[op.name] < 0x20


_register(HATX)

F32 = mybir.dt.float32
F32R = mybir.dt.float32r
ALU = mybir.AluOpType
BF16 = mybir.dt.bfloat16

N = 4096
NCORES = 8
ROWS = N // NCORES   # 512
HALF = 256
NH = 2
DMIN, DMAX = -5, 7   # band d = j - i; must cover (off_min-2, off_max+2)
ND = DMAX - DMIN + 1           # 13
SMIN, SMAX = -2, 8             # alive rows need k + d in [SMIN, SMAX]
PLIST = [(k, di, h) for k in (1, 2, 3) for di in range(ND)
         for h in range(NH) if SMIN <= k + DMIN + di <= SMAX]
NP = len(PLIST)                # 66 output-partition rows (k, d, h)
P_OF = {t: i for i, t in enumerate(PLIST)}
NR = ND * 3 * NH + 2           # 80 conv rows (d, e, h) + ones + pad
C_WL = HALF                    # PK2 col of W column      [NP, 2]
C_S0 = HALF + 2                # PK2 col of hat scalar s0
C_S1 = HALF + 3                # PK2 col of hat scalar s1
W2 = HALF + 4                  # PK2 width (260, even)
W3 = NP + 2                    # PK3 width (80, even)
NWARM = 8                      # PE p-state warmup matmuls


def _emit(tc, nc, pk13_d, pk2_d, y_d):
    with (
        tc.tile_pool(name="const", bufs=1) as const,
        tc.tile_pool(name="work", bufs=1) as work,
        tc.tile_pool(name="psum", bufs=1, space="PSUM") as psum,
    ):
        # PE p-state warmup: dummy matmuls with no input deps run during
        # the input-DMA wait and ramp the tensor-engine clock.  The zero
        # tile is memset on GpSimd (idle) so warmups start ASAP.
        wz = const.tile([128, 258], F32R)
        nc.gpsimd.memset(wz[:].bitcast(F32), 0.0)
        psW = psum.tile([2, HALF], F32, tag="psW")
        for _ in range(NWARM):
            nc.tensor.matmul(psW[:], wz[:, 0:2], wz[:, 2:258],
                             start=True, stop=True, skip_group_check=True)

        # conv operands travel as bf16 (halves the big DMA payloads; off
        # error ~3e-3 rel, well under the 2e-2 gate).  Merging rhs+lhsT
        # into one sub-1KB-row DMA makes every input first on its queue.
        PK13 = const.tile([NR, HALF + W3], BF16)
        nc.scalar.dma_start(PK13[:], pk13_d[:, :].bitcast(BF16))
        PK2 = const.tile([NP, W2], F32R)
        nc.sync.dma_start(PK2[:], pk2_d[:, :].bitcast(F32R))

        xv = PK2[:, 0:HALF].bitcast(F32)
        s0 = PK2[:, C_S0:C_S0 + 1].bitcast(F32)
        s1 = PK2[:, C_S1:C_S1 + 1].bitcast(F32)

        psC = psum.tile([NP, HALF], F32, tag="psC")
        nc.tensor.matmul(psC[:], PK13[:, HALF:HALF + NP], PK13[:, 0:HALF],
                         start=True, stop=True)

        H = work.tile([NP, HALF], F32R, tag="H")
        nc.vector._custom_dve(HATX, out=H[:], in0=psC[:], in1=xv,
                              s0=s0, s1=s1)

        psY = psum.tile([NH, HALF], F32, tag="psY")
        nc.tensor.matmul(psY[:], PK2[:, C_WL:C_WL + NH], H[:],
                         start=True, stop=True)

        ysb = work.tile([NH, HALF], F32, tag="ysb")
        nc.vector.tensor_scalar(ysb[:], psY[:], 0.0, None, ALU.add)
        nc.scalar.dma_start(y_d[:, :], ysb[:, :])


_CACHE = {}


def build():
    if "nc" in _CACHE:
        return _CACHE["nc"]
    nc = bacc.Bacc("TRN2", target_bir_lowering=False, debug=False)
    pk13 = nc.dram_tensor("pk13", [NR, (HALF + W3) // 2], F32,
                          kind="ExternalInput").ap()
    pk2 = nc.dram_tensor("pk2", [NP, W2], F32, kind="ExternalInput").ap()
    y = nc.dram_tensor("y", [NH, HALF], F32, kind="ExternalOutput").ap()
    with tile.TileContext(nc) as tc:
        _emit(tc, nc, pk13, pk2, y)
    nc.compile()
    _CACHE["nc"] = nc
    return nc


def make_in_maps(x, conv_w, conv_b, W):
    xf = np.ascontiguousarray(x, dtype=np.float32).reshape(-1)
    assert xf.shape[0] == N
    cw = np.asarray(conv_w, dtype=np.float32).reshape(-1)
    cb = float(np.asarray(conv_b, dtype=np.float32).reshape(-1)[0])
    Wf = np.asarray(W, dtype=np.float32).reshape(-1)

    # sanity: the hardcoded band covers the actual offset range
    xb = xf.reshape(4, 1024)
    pad = np.pad(xb, ((0, 0), (1, 1)))
    conv = cw[0] * pad[:, :-2] + cw[1] * pad[:, 1:-1] + cw[2] * pad[:, 2:] + cb
    off = (np.maximum(conv, 0) - xb).reshape(-1)
    assert off.min() > DMIN - 2 and off.max() < DMAX + 2, (
        f"off range [{off.min()}, {off.max()}] outside band ({DMIN},{DMAX})")
    # rows with k+d outside [SMIN, SMAX] have identically-zero hats
    assert off.min() >= SMIN - 2 and off.max() <= SMAX - 2, (
        f"off range [{off.min()}, {off.max()}] breaks alive-row pruning")

    PAD = 32
    jj = np.arange(N)
    val = {}
    for e in (-1, 0, 1):
        v = np.zeros(N + 2 * PAD, dtype=np.float32)
        ok = ((jj % 1024) + e >= 0) & ((jj % 1024) + e < 1024)
        v[jj + PAD] = np.where(ok, xf[np.clip(jj + e, 0, N - 1)], 0.0)
        val[e] = v
    xun = np.zeros(N + 2 * PAD, dtype=np.float32)
    xun[PAD:PAD + N] = xf

    def r_of(di, e, h):
        return (di * 3 + (e + 1)) * NH + h

    CW = np.zeros((NR, NP), dtype=np.float32)
    for (k, di, h) in PLIST:
        for e in (-1, 0, 1):
            CW[r_of(di, e, h), P_OF[(k, di, h)]] = cw[e + 1]
    CW[NR - 2, :] = cb          # ones row; NR-1 is zero padding

    import ml_dtypes
    CWb = CW.astype(ml_dtypes.bfloat16)
    wls = np.zeros((NP, 4), dtype=np.float32)
    for (k, di, h) in PLIST:
        p = P_OF[(k, di, h)]
        wls[p, h] = Wf[3 - k]
        wls[p, 2] = 3 - k - (DMIN + di)
        wls[p, 3] = k - 1 + (DMIN + di)

    in_maps = []
    for d in range(NCORES):
        pk1b = np.zeros((NR, HALF + W3), dtype=ml_dtypes.bfloat16)
        pk1b[:, HALF:HALF + NP] = CWb
        pk2 = np.zeros((NP, W2), dtype=np.float32)
        pk2[:, HALF:HALF + 4] = wls
        for di in range(ND):
            dlt = DMIN + di
            for h in range(NH):
                j0 = PAD + 512 * d + HALF * h + dlt
                for e in (-1, 0, 1):
                    pk1b[r_of(di, e, h), 0:HALF] = val[e][j0:j0 + HALF].astype(
                        ml_dtypes.bfloat16)
                for k in (1, 2, 3):
                    if (k, di, h) in P_OF:
                        pk2[P_OF[(k, di, h)], 0:HALF] = xun[j0:j0 + HALF]
        pk1b[NR - 2, 0:HALF] = 1.0
        in_maps.append({"pk13": pk1b.view(np.float32), "pk2": pk2})
    return in_maps


def run(x, conv_w, conv_b, W, trace=False, **kw):
    nc = build()
    in_maps = make_in_maps(x, conv_w, conv_b, W)
    res = run_bass_kernel_spmd(
        nc, in_maps, core_ids=list(range(NCORES)), trace=trace, **kw)
    y = np.concatenate([res.results[d]["y"].ravel() for d in range(NCORES)])
    return y.reshape(np.asarray(x).shape).astype(np.float32), res


def kernel(x, conv_w, conv_b, W):
    y, _ = run(x, conv_w, conv_b, W)
    return y


# revision 26
# speedup vs baseline: 1.1087x; 1.1087x over previous
"""Deformable Conv1D kernel v12 for Trainium2 (8 NeuronCores, Bass/Tile).

Diagonal (banded) layout. y[i] = sum_{d,k} W[3-k] * x[i+d] *
hat(off[i+d] + (2 - d) - k), where off = relu(conv1d(x)) - x and
hat(u) = relu(min(u+1, 1-u)); the band d = j - i is in [DMIN, DMAX]
(covers supp of the hat for the actual off range, asserted on host).

Per core (512 outputs, split into 2 halves h of 256):
  rows p = (k, d, h), NP = 3*ND*2 = 78 partitions; free dim = ii (256).

  psC[p,ii] = conv(x)[j],  j = 512*core + 256*h + d + ii     (one PE matmul:
              lhsT = block conv weights [NR, NP], rhs = shifted/batch-masked
              x rows [NR, 256], both bf16)
  H[p,ii]   = x[j] * relu(min(off + s0[p], s1[p] - off)),
              off = relu(psC) - x[j]                          (ONE fused
              custom-DVE uop straight from PSUM; s0 = 3-k-d, s1 = k-1+d)
  psY[h,ii] = sum_p WL[p,h] * H[p,ii]                         (one PE matmul,
              f32r full-rate at 256 cols; WL[p,h'] = W[3-k(p)]*[h(p)==h'])
  y         = psY -> SBUF copy -> DMA out.

Input schedule: two DMAs, each FIRST on its own HWDGE queue (the second
DMA on a queue starts its packets ~1us later): PK13 = conv rhs+lhsT in
bf16, 672B rows, on the Sync queue; PK2 = fp32 x values + W column +
hat scalars, 1040B rows, on the Activation queue.  Rows are kept
<= 1KB — larger rows collapse onto a single SDMA engine and serialize.
Dummy matmuls with no input deps run during the DMA wait to ramp the
PE p-state.  Everything else (preamble barriers, DMA trigger latency
~1.5us, and the NRT-injected 253-semaphore clear epilogue ~6us) is
fixed harness overhead.
"""

import sys

for _p in ("/opt/trn_rl_repo",):
    if _p not in sys.path:
        sys.path.insert(0, _p)

import numpy as np

import concourse.bass as bass
import concourse.tile as tile
from concourse import bacc, mybir
from concourse import dve_ops as _dve_ops
from concourse.bass_utils import run_bass_kernel_spmd
from concourse.dve_ops import DveOp
from concourse.dve_spec import C0, C1, Spec, Src0, Src1, minn, relu

# Fused custom-DVE op (single uop):
#   off = relu(in0) - in1;  out = in1 * relu(min(off + s0, s1 - off))
_r = relu(Src0)
_t = _r - Src1
HATX = DveOp(
    "HATX",
    Spec(
        body=Src1 * relu(minn(_t + C0, C1 - _t)),
        reference=lambda in0, in1, s0, s1, imm2: (
            lambda off: (
                in1 * np.maximum(np.minimum(off + s0, s1 - off), 0)
            ).astype(np.float32)
        )(np.maximum(in0, 0) - in1),
    ),
    subdim=False,
    uops_sha={"v3": "355cfd13d7758520", "v4": "aaff841bf6e5b216"},
)


def _register(op):
    if op.name not in _dve_ops._SUB_OPCODE_FOR_NAME:
        _dve_ops.OPS.append(op)
        _dve_ops.CUSTOM_DVE_SPECS[op.name] = op.spec
        _dve_ops._SUB_OPCODE_FOR_NAME[op.name] = (
            max(_dve_ops._SUB_OPCODE_FOR_NAME.values()) + 1)
        assert _dve_ops._SUB_OPCODE_FOR_NAME[op.name] < 0x20


_register(HATX)

F32 = mybir.dt.float32
F32R = mybir.dt.float32r
ALU = mybir.AluOpType
BF16 = mybir.dt.bfloat16

N = 4096
NCORES = 8
ROWS = N // NCORES   # 512
HALF = 256
NH = 2
DMIN, DMAX = -5, 7   # band d = j - i; must cover (off_min-2, off_max+2)
ND = DMAX - DMIN + 1           # 13
SMIN, SMAX = -2, 8             # alive rows need k + d in [SMIN, SMAX]
PLIST = [(k, di, h) for k in (1, 2, 3) for di in range(ND)
         for h in range(NH) if SMIN <= k + DMIN + di <= SMAX]
NP = len(PLIST)                # 66 output-partition rows (k, d, h)
P_OF = {t: i for i, t in enumerate(PLIST)}
NR = ND * 3 * NH + 2           # 80 conv rows (d, e, h) + ones + pad
C_WL = HALF                    # PK2 col of W column      [NP, 2]
C_S0 = HALF + 2                # PK2 col of hat scalar s0
C_S1 = HALF + 3                # PK2 col of hat scalar s1
W2 = HALF + 4                  # PK2 width (260, even)
W3 = NP + 2                    # PK3 width (80, even)
NWARM = 8                      # PE p-state warmup matmuls


def _emit(tc, nc, pk13_d, pk2_d, y_d):
    with (
        tc.tile_pool(name="const", bufs=1) as const,
        tc.tile_pool(name="work", bufs=1) as work,
        tc.tile_pool(name="psum", bufs=1, space="PSUM") as psum,
    ):
        # PE p-state warmup: dummy matmuls with no input deps run during
        # the input-DMA wait and ramp the tensor-engine clock.  The zero
        # tile is memset on GpSimd (idle) so warmups start ASAP.
        wz = const.tile([128, 258], F32R)
        nc.gpsimd.memset(wz[:].bitcast(F32), 0.0)
        psW = psum.tile([2, HALF], F32, tag="psW")
        for _ in range(NWARM):
            nc.tensor.matmul(psW[:], wz[:, 0:2], wz[:, 2:258],
                             start=True, stop=True, skip_group_check=True)

        # conv operands travel as bf16 (halves the big DMA payloads; off
        # error ~3e-3 rel, well under the 2e-2 gate).  Merging rhs+lhsT
        # into one sub-1KB-row DMA makes every input first on its queue.
        PK13 = const.tile([NR, HALF + W3], BF16)
        nc.sync.dma_start(PK13[:], pk13_d[:, :].bitcast(BF16))
        PK2 = const.tile([NP, W2], F32R)
        nc.scalar.dma_start(PK2[:], pk2_d[:, :].bitcast(F32R))

        xv = PK2[:, 0:HALF].bitcast(F32)
        s0 = PK2[:, C_S0:C_S0 + 1].bitcast(F32)
        s1 = PK2[:, C_S1:C_S1 + 1].bitcast(F32)

        psC = psum.tile([NP, HALF], F32, tag="psC")
        nc.tensor.matmul(psC[:], PK13[:, HALF:HALF + NP], PK13[:, 0:HALF],
                         start=True, stop=True)

        H = work.tile([NP, HALF], F32R, tag="H")
        nc.vector._custom_dve(HATX, out=H[:], in0=psC[:], in1=xv,
                              s0=s0, s1=s1)

        psY = psum.tile([NH, HALF], F32, tag="psY")
        nc.tensor.matmul(psY[:], PK2[:, C_WL:C_WL + NH], H[:],
                         start=True, stop=True)

        ysb = work.tile([NH, HALF], F32, tag="ysb")
        nc.vector.tensor_scalar(ysb[:], psY[:], 0.0, None, ALU.add)
        nc.scalar.dma_start(y_d[:, :], ysb[:, :])


_CACHE = {}


def build():
    if "nc" in _CACHE:
        return _CACHE["nc"]
    nc = bacc.Bacc("TRN2", target_bir_lowering=False, debug=False)
    pk13 = nc.dram_tensor("pk13", [NR, (HALF + W3) // 2], F32,
                          kind="ExternalInput").ap()
    pk2 = nc.dram_tensor("pk2", [NP, W2], F32, kind="ExternalInput").ap()
    y = nc.dram_tensor("y", [NH, HALF], F32, kind="ExternalOutput").ap()
    with tile.TileContext(nc) as tc:
        _emit(tc, nc, pk13, pk2, y)
    nc.compile()
    _CACHE["nc"] = nc
    return nc


def make_in_maps(x, conv_w, conv_b, W):
    xf = np.ascontiguousarray(x, dtype=np.float32).reshape(-1)
    assert xf.shape[0] == N
    cw = np.asarray(conv_w, dtype=np.float32).reshape(-1)
    cb = float(np.asarray(conv_b, dtype=np.float32).reshape(-1)[0])
    Wf = np.asarray(W, dtype=np.float32).reshape(-1)

    # sanity: the hardcoded band covers the actual offset range
    xb = xf.reshape(4, 1024)
    pad = np.pad(xb, ((0, 0), (1, 1)))
    conv = cw[0] * pad[:, :-2] + cw[1] * pad[:, 1:-1] + cw[2] * pad[:, 2:] + cb
    off = (np.maximum(conv, 0) - xb).reshape(-1)
    assert off.min() > DMIN - 2 and off.max() < DMAX + 2, (
        f"off range [{off.min()}, {off.max()}] outside band ({DMIN},{DMAX})")
    # rows with k+d outside [SMIN, SMAX] have identically-zero hats
    assert off.min() >= SMIN - 2 and off.max() <= SMAX - 2, (
        f"off range [{off.min()}, {off.max()}] breaks alive-row pruning")

    PAD = 32
    jj = np.arange(N)
    val = {}
    for e in (-1, 0, 1):
        v = np.zeros(N + 2 * PAD, dtype=np.float32)
        ok = ((jj % 1024) + e >= 0) & ((jj % 1024) + e < 1024)
        v[jj + PAD] = np.where(ok, xf[np.clip(jj + e, 0, N - 1)], 0.0)
        val[e] = v
    xun = np.zeros(N + 2 * PAD, dtype=np.float32)
    xun[PAD:PAD + N] = xf

    def r_of(di, e, h):
        return (di * 3 + (e + 1)) * NH + h

    CW = np.zeros((NR, NP), dtype=np.float32)
    for (k, di, h) in PLIST:
        for e in (-1, 0, 1):
            CW[r_of(di, e, h), P_OF[(k, di, h)]] = cw[e + 1]
    CW[NR - 2, :] = cb          # ones row; NR-1 is zero padding

    import ml_dtypes
    CWb = CW.astype(ml_dtypes.bfloat16)
    wls = np.zeros((NP, 4), dtype=np.float32)
    for (k, di, h) in PLIST:
        p = P_OF[(k, di, h)]
        wls[p, h] = Wf[3 - k]
        wls[p, 2] = 3 - k - (DMIN + di)
        wls[p, 3] = k - 1 + (DMIN + di)

    in_maps = []
    for d in range(NCORES):
        pk1b = np.zeros((NR, HALF + W3), dtype=ml_dtypes.bfloat16)
        pk1b[:, HALF:HALF + NP] = CWb
        pk2 = np.zeros((NP, W2), dtype=np.float32)
        pk2[:, HALF:HALF + 4] = wls
        for di in range(ND):
            dlt = DMIN + di
            for h in range(NH):
                j0 = PAD + 512 * d + HALF * h + dlt
                for e in (-1, 0, 1):
                    pk1b[r_of(di, e, h), 0:HALF] = val[e][j0:j0 + HALF].astype(
                        ml_dtypes.bfloat16)
                for k in (1, 2, 3):
                    if (k, di, h) in P_OF:
                        pk2[P_OF[(k, di, h)], 0:HALF] = xun[j0:j0 + HALF]
        pk1b[NR - 2, 0:HALF] = 1.0
        in_maps.append({"pk13": pk1b.view(np.float32), "pk2": pk2})
    return in_maps


def run(x, conv_w, conv_b, W, trace=False, **kw):
    nc = build()
    in_maps = make_in_maps(x, conv_w, conv_b, W)
    res = run_bass_kernel_spmd(
        nc, in_maps, core_ids=list(range(NCORES)), trace=trace, **kw)
    y = np.concatenate([res.results[d]["y"].ravel() for d in range(NCORES)])
    return y.reshape(np.asarray(x).shape).astype(np.float32), res


def kernel(x, conv_w, conv_b, W):
    y, _ = run(x, conv_w, conv_b, W)
    return y
